# revision 17
# baseline (speedup 1.0000x reference)
"""MLAttention (label-pooling attention) Trainium2 Bass kernel.

Computes, for full inputs:
    scores = einsum('bsh,lh->bls', inputs, W)
    scores = where(mask==0, -inf, scores)
    attn   = softmax(scores, axis=-1)
    out    = einsum('bls,bsh->blh', attn, inputs)

Label-parallel across 8 NeuronCores: L=28415 padded to 28672 = 8*3584.
Each core gets its own W shard [3584, 512]; inputs/masks replicated.
Host concatenates the 8 per-core outputs [B, 3584, H] and trims to L.

Per-core dataflow (all fp32):
  setup:  inputs -> SBUF (natural [S,H] chunks) and PE-transposed [H,S]
          chunks; W shard PE-transposed once into WT[h, l] resident in
          SBUF; mask row broadcast to 128 partitions via K=1 matmul.
  main:   per (b, 128-label tile):
            PE   : scores_psum = sum_k WT_k^T @ XT_k      (4 matmuls N=512)
            ACT  : exp_sbuf    = Exp(scores_psum)
            DVE  : expm = exp*maskrep, rowsum (one scalar_tensor_tensor)
            DVE  : recip = 1/rowsum
            PE   : expT_psum   = transpose(expm)           (4 transposes)
            DVE  : expT_sbuf   = copy(expT_psum)
            PE   : out_psum    = sum_s expT_s^T @ X_s      (4 matmuls N=512)
            ACT  : out_sbuf    = Copy(out_psum * recip)    (per-label scale)
            DMA  : out_sbuf -> out[b, tile, :]
"""

from contextlib import ExitStack

import numpy as np

import concourse.bass as bass
import concourse.mybir as mybir
import concourse.tile as tile
from concourse import bacc, bass_utils
from concourse.bass import ts
from concourse.masks import make_identity

F32 = mybir.dt.float32

# Problem shapes (hardcoded per contract).
B, S, H, L = 4, 512, 512, 28415
N_CORES = 8
LSH = 3584               # per-core padded label count (28 tiles of 128)
L_PAD = LSH * N_CORES    # 28672


def build_module(b_sz=B, s_sz=S, h_sz=H, lsh=LSH, n_devices=N_CORES, mm_dt=None):
    """Build the per-core Bass/Tile module (SPMD: same program, per-core data)."""
    if mm_dt is None:
        mm_dt = mybir.dt.float32r  # full-rate PE mode for N>=256 fp32 matmuls
    P = 128
    KH = h_sz // P   # H contraction chunks
    KS = s_sz // P   # S contraction chunks
    NT = lsh // P    # label tiles per core

    nc = bacc.Bacc(
        "TRN2", target_bir_lowering=False, debug=False, num_devices=n_devices
    )
    x_d = nc.dram_tensor("x", [b_sz, s_sz, h_sz], F32, kind="ExternalInput").ap()
    w_d = nc.dram_tensor("w", [lsh, h_sz], F32, kind="ExternalInput").ap()
    m_d = nc.dram_tensor("m", [b_sz, s_sz], F32, kind="ExternalInput").ap()
    o_d = nc.dram_tensor("o", [b_sz, lsh, h_sz], F32, kind="ExternalOutput").ap()

    with tile.TileContext(nc) as tc, ExitStack() as ctx:
        const = ctx.enter_context(tc.tile_pool(name="const", bufs=1))
        res = ctx.enter_context(tc.tile_pool(name="res", bufs=1))
        work = ctx.enter_context(tc.tile_pool(name="work", bufs=3))
        psum = ctx.enter_context(tc.tile_pool(name="psum", bufs=2, space="PSUM"))

        F16 = mybir.dt.float16

        ident = const.tile([P, P], F32)
        make_identity(nc, ident[:])
        ident_h = const.tile([P, P], F16)
        nc.vector.tensor_copy(ident_h[:], ident[:])
        zbias = const.tile([P, 1], F32)
        nc.gpsimd.memset(zbias[:], 0.0)
        ones_row = const.tile([1, P], F32)
        nc.gpsimd.memset(ones_row[:], 1.0)

        # Resident SBUF tensors. Matmul operands are float32r (same bits as
        # fp32; the producing copy rounds) so the PE runs single-pass
        # full-rate matmuls instead of fp32's 2x half-speed passes.
        WT = res.tile([P, KH, lsh], mm_dt)        # WT[h%128, h//128, l] = W[l, h]
        XB = res.tile([P, b_sz, KS, h_sz], mm_dt)  # XB[s%128, b, s//128, h]
        XT = res.tile([P, b_sz, KH, s_sz], mm_dt)  # XT[h%128, b, h//128, s]
        MR = res.tile([P, b_sz, s_sz], F32)      # mask row replicated over partitions

        def x_setup(b):
            """Stage inputs[b], round into XB, PE-transpose into XT."""
            xstage = work.tile([P, KS, h_sz], F32, tag="xstage", bufs=2)
            nc.sync.dma_start(
                xstage[:], x_d[b].rearrange("(c p) h -> p c h", p=P)
            )
            nc.vector.tensor_copy(XB[:, b], xstage[:])
            for c in range(KS):
                pt = psum.tile([P, KH, P], F32, tag="ps_tx")
                for k in range(KH):
                    nc.tensor.transpose(
                        pt[:, k, :], xstage[:, c, ts(k, P)], ident[:]
                    )
                nc.vector.tensor_copy(XT[:, b, :, ts(c, P)], pt[:])

        def mask_setup(b):
            """Replicate mask row across partitions via K=1 matmul with ones."""
            mrow = work.tile([1, s_sz], F32, tag="mrow")
            nc.sync.dma_start(mrow[:], m_d[b : b + 1, :])
            pm = psum.tile([P, s_sz], F32, tag="ps_sc", bufs=2)
            nc.tensor.matmul(pm[:], ones_row[:], mrow[:], start=True, stop=True)
            nc.vector.tensor_copy(MR[:, b, :], pm[:])

        def w_setup(t):
            """Load + PE-transpose one 128-label W tile into WT."""
            wtile = work.tile([P, h_sz], F32, tag="wload")
            nc.sync.dma_start(wtile[:], w_d[ts(t, P), :])
            pt = psum.tile([P, KH, P], F32, tag="ps_tx")
            for k in range(KH):
                nc.tensor.transpose(pt[:, k, :], wtile[:, ts(k, P)], ident[:])
            nc.vector.tensor_copy(WT[:, :, ts(t, P)], pt[:])

        def main_tile(b, t):
            ps_sc = psum.tile([P, s_sz], F32, tag="ps_sc", bufs=2)
            for k in range(KH):
                nc.tensor.matmul(
                    ps_sc[:],
                    WT[:, k, ts(t, P)],
                    XT[:, b, k, :],
                    start=(k == 0),
                    stop=(k == KH - 1),
                )

            exp_t = work.tile([P, s_sz], F32, tag="exp")
            nc.scalar.activation(
                exp_t[:], ps_sc[:], mybir.ActivationFunctionType.Exp,
                bias=zbias[:],
            )

            # Mask + row-sum in one DVE pass; expm in fp16 so the PE
            # transposes get a 2-byte stationary load (half the LDW cost).
            expm = work.tile([P, s_sz], F16, tag="expm")
            rowsum = work.tile([P, 1], F32, tag="rowsum")
            nc.vector.scalar_tensor_tensor(
                out=expm[:],
                in0=exp_t[:],
                scalar=1.0,
                in1=MR[:, b, :],
                op0=mybir.AluOpType.mult,
                op1=mybir.AluOpType.mult,
                accum_out=rowsum[:],
            )
            recip = work.tile([P, 1], F32, tag="recip")
            nc.vector.reciprocal(recip[:], rowsum[:])

            ps_tr = psum.tile([P, KS, P], F16, tag="ps_tr")
            for c in range(KS):
                nc.tensor.transpose(
                    ps_tr[:, c, :], expm[:, ts(c, P)], ident_h[:]
                )
            expT = work.tile([P, KS, P], mm_dt, tag="expT")
            nc.vector.tensor_copy(expT[:], ps_tr[:])

            ps_out = psum.tile([P, h_sz], F32, tag="ps_out")
            for c in range(KS):
                nc.tensor.matmul(
                    ps_out[:],
                    expT[:, c, :],
                    XB[:, b, c, :],
                    start=(c == 0),
                    stop=(c == KS - 1),
                )

            out_t = work.tile([P, h_sz], F32, tag="out")
            nc.scalar.activation(
                out_t[:], ps_out[:], mybir.ActivationFunctionType.Copy,
                scale=recip[:],
            )
            nc.sync.dma_start(o_d[b, ts(t, P), :], out_t[:])

        # ---- emission order tuned for DMA pipelining + PE warm-up:
        # mask + b=0 inputs first, then the b=0 label pass with W tile
        # loads fused in (each W tile DMA overlaps the previous tile's
        # compute), then the remaining batches (inputs DMA'd during the
        # b=0 pass).
        for b in range(b_sz):
            mask_setup(b)
        x_setup(0)
        for t in range(NT):
            w_setup(t)
            main_tile(0, t)
        for b in range(1, b_sz):
            x_setup(b)
        for b in range(1, b_sz):
            for t in range(NT):
                main_tile(b, t)

    nc.compile()
    return nc


_CACHE = {}


def _get_module():
    if "nc" not in _CACHE:
        _CACHE["nc"] = build_module()
    return _CACHE["nc"]


def _run(inputs: np.ndarray, masks: np.ndarray, W: np.ndarray, **spmd_kwargs):
    """Run on 8 cores; returns (full output, BassKernelResults)."""
    nc = _get_module()

    x = np.ascontiguousarray(inputs, dtype=np.float32)
    mf = np.ascontiguousarray(masks, dtype=np.float32)
    w_pad = np.zeros((L_PAD, H), dtype=np.float32)
    w_pad[:L] = W

    in_maps = [
        {"x": x, "m": mf, "w": np.ascontiguousarray(w_pad[c * LSH : (c + 1) * LSH])}
        for c in range(N_CORES)
    ]
    res = bass_utils.run_bass_kernel_spmd(
        nc, in_maps, core_ids=list(range(N_CORES)), **spmd_kwargs
    )
    out = np.concatenate([res.results[c]["o"] for c in range(N_CORES)], axis=1)
    return np.ascontiguousarray(out[:, :L, :]), res


def kernel(inputs: np.ndarray, masks: np.ndarray, W: np.ndarray) -> np.ndarray:
    out, _ = _run(inputs, masks, W)
    return out


# revision 18
# speedup vs baseline: 1.1640x; 1.1640x over previous
"""MLAttention (label-pooling attention) Trainium2 Bass kernel.

Computes, for full inputs:
    scores = einsum('bsh,lh->bls', inputs, W)
    scores = where(mask==0, -inf, scores)
    attn   = softmax(scores, axis=-1)
    out    = einsum('bls,bsh->blh', attn, inputs)

Label-parallel across 8 NeuronCores: L=28415 padded to 28672 = 8*3584.
Each core gets its own W shard [3584, 512]; inputs/masks replicated.
Host concatenates the 8 per-core outputs [B, 3584, H] and trims to L.

Per-core dataflow (all fp32):
  setup:  inputs -> SBUF (natural [S,H] chunks) and PE-transposed [H,S]
          chunks; W shard PE-transposed once into WT[h, l] resident in
          SBUF; mask row broadcast to 128 partitions via K=1 matmul.
  main:   per (b, 128-label tile):
            PE   : scores_psum = sum_k WT_k^T @ XT_k      (4 matmuls N=512)
            ACT  : exp_sbuf    = Exp(scores_psum)
            DVE  : expm = exp*maskrep, rowsum (one scalar_tensor_tensor)
            DVE  : recip = 1/rowsum
            PE   : expT_psum   = transpose(expm)           (4 transposes)
            DVE  : expT_sbuf   = copy(expT_psum)
            PE   : out_psum    = sum_s expT_s^T @ X_s      (4 matmuls N=512)
            ACT  : out_sbuf    = Copy(out_psum * recip)    (per-label scale)
            DMA  : out_sbuf -> out[b, tile, :]
"""

from contextlib import ExitStack

import numpy as np

import concourse.bass as bass
import concourse.mybir as mybir
import concourse.tile as tile
from concourse import bacc, bass_utils
from concourse.bass import ts
from concourse.masks import make_identity

F32 = mybir.dt.float32

# Problem shapes (hardcoded per contract).
B, S, H, L = 4, 512, 512, 28415
N_CORES = 8
LSH = 3584               # per-core padded label count (28 tiles of 128)
L_PAD = LSH * N_CORES    # 28672


def build_module(b_sz=B, s_sz=S, h_sz=H, lsh=LSH, n_devices=N_CORES, mm_dt=None):
    """Build the per-core Bass/Tile module (SPMD: same program, per-core data)."""
    if mm_dt is None:
        mm_dt = mybir.dt.float32r  # full-rate PE mode for N>=256 fp32 matmuls
    P = 128
    KH = h_sz // P   # H contraction chunks
    KS = s_sz // P   # S contraction chunks
    NT = lsh // P    # label tiles per core

    nc = bacc.Bacc(
        "TRN2", target_bir_lowering=False, debug=False, num_devices=n_devices
    )
    x_d = nc.dram_tensor("x", [b_sz, s_sz, h_sz], F32, kind="ExternalInput").ap()
    w_d = nc.dram_tensor("w", [lsh, h_sz], F32, kind="ExternalInput").ap()
    m_d = nc.dram_tensor("m", [b_sz, s_sz], F32, kind="ExternalInput").ap()
    o_d = nc.dram_tensor("o", [b_sz, lsh, h_sz], F32, kind="ExternalOutput").ap()

    with tile.TileContext(nc) as tc, ExitStack() as ctx:
        const = ctx.enter_context(tc.tile_pool(name="const", bufs=1))
        res = ctx.enter_context(tc.tile_pool(name="res", bufs=1))
        work = ctx.enter_context(tc.tile_pool(name="work", bufs=3))
        psum = ctx.enter_context(tc.tile_pool(name="psum", bufs=2, space="PSUM"))

        ident = const.tile([P, P], F32)
        make_identity(nc, ident[:])
        ident_r = const.tile([P, P], mm_dt)
        nc.vector.tensor_copy(ident_r[:], ident[:])
        zbias = const.tile([P, 1], F32)
        nc.gpsimd.memset(zbias[:], 0.0)
        ones_row = const.tile([1, P], F32)
        nc.gpsimd.memset(ones_row[:], 1.0)

        # Resident SBUF tensors. Matmul operands are float32r (same bits as
        # fp32; the producing copy rounds) so the PE runs single-pass
        # full-rate matmuls instead of fp32's 2x half-speed passes.
        WT = res.tile([P, KH, lsh], mm_dt)        # WT[h%128, h//128, l] = W[l, h]
        XB = res.tile([P, b_sz, KS, h_sz], mm_dt)  # XB[s%128, b, s//128, h]
        XT = res.tile([P, b_sz, KH, s_sz], mm_dt)  # XT[h%128, b, h//128, s]
        MR = res.tile([P, b_sz, s_sz], F32)      # mask row replicated over partitions

        def x_setup(b):
            """Stage inputs[b], round into XB, PE-transpose into XT."""
            xstage = work.tile([P, KS, h_sz], F32, tag="xstage", bufs=2)
            nc.sync.dma_start(
                xstage[:], x_d[b].rearrange("(c p) h -> p c h", p=P)
            )
            nc.vector.tensor_copy(XB[:, b], xstage[:])
            for c in range(KS):
                pt = psum.tile([P, KH, P], F32, tag="ps_tx")
                for k in range(KH):
                    nc.tensor.transpose(
                        pt[:, k, :], xstage[:, c, ts(k, P)], ident[:]
                    )
                nc.vector.tensor_copy(XT[:, b, :, ts(c, P)], pt[:])

        def mask_setup(b):
            """Replicate mask row across partitions via K=1 matmul with ones."""
            mrow = work.tile([1, s_sz], F32, tag="mrow")
            nc.sync.dma_start(mrow[:], m_d[b : b + 1, :])
            pm = psum.tile([P, s_sz], F32, tag="ps_sc", bufs=2)
            nc.tensor.matmul(pm[:], ones_row[:], mrow[:], start=True, stop=True)
            nc.vector.tensor_copy(MR[:, b, :], pm[:])

        def w_setup(t):
            """Load + PE-transpose one 128-label W tile into WT."""
            wtile = work.tile([P, h_sz], F32, tag="wload")
            nc.sync.dma_start(wtile[:], w_d[ts(t, P), :])
            pt = psum.tile([P, KH, P], F32, tag="ps_tx")
            for k in range(KH):
                nc.tensor.transpose(pt[:, k, :], wtile[:, ts(k, P)], ident[:])
            nc.vector.tensor_copy(WT[:, :, ts(t, P)], pt[:])

        def main_tile(b, t):
            ps_sc = psum.tile([P, s_sz], F32, tag="ps_sc", bufs=2)
            for k in range(KH):
                nc.tensor.matmul(
                    ps_sc[:],
                    WT[:, k, ts(t, P)],
                    XT[:, b, k, :],
                    start=(k == 0),
                    stop=(k == KH - 1),
                )

            exp_t = work.tile([P, s_sz], F32, tag="exp")
            nc.scalar.activation(
                exp_t[:], ps_sc[:], mybir.ActivationFunctionType.Exp,
                bias=zbias[:],
            )

            # Mask + row-sum in one DVE pass.
            expm = work.tile([P, s_sz], mm_dt, tag="expm")
            rowsum = work.tile([P, 1], F32, tag="rowsum")
            nc.vector.scalar_tensor_tensor(
                out=expm[:],
                in0=exp_t[:],
                scalar=1.0,
                in1=MR[:, b, :],
                op0=mybir.AluOpType.mult,
                op1=mybir.AluOpType.mult,
                accum_out=rowsum[:],
            )
            recip = work.tile([P, 1], F32, tag="recip")
            nc.vector.reciprocal(recip[:], rowsum[:])

            ps_tr = psum.tile([P, KS, P], mm_dt, tag="ps_tr")
            for c in range(KS):
                nc.tensor.transpose(
                    ps_tr[:, c, :], expm[:, ts(c, P)], ident_r[:]
                )
            expT = work.tile([P, KS, P], mm_dt, tag="expT")
            nc.vector.tensor_copy(expT[:], ps_tr[:])

            ps_out = psum.tile([P, h_sz], F32, tag="ps_out")
            for c in range(KS):
                nc.tensor.matmul(
                    ps_out[:],
                    expT[:, c, :],
                    XB[:, b, c, :],
                    start=(c == 0),
                    stop=(c == KS - 1),
                )

            out_t = work.tile([P, h_sz], F32, tag="out")
            nc.scalar.activation(
                out_t[:], ps_out[:], mybir.ActivationFunctionType.Copy,
                scale=recip[:],
            )
            nc.sync.dma_start(o_d[b, ts(t, P), :], out_t[:])

        # ---- emission order tuned for DMA pipelining + PE warm-up:
        # mask + b=0 inputs first, then the b=0 label pass with W tile
        # loads fused in (each W tile DMA overlaps the previous tile's
        # compute), then the remaining batches (inputs DMA'd during the
        # b=0 pass).
        for b in range(b_sz):
            mask_setup(b)
        x_setup(0)
        for t in range(NT):
            w_setup(t)
            main_tile(0, t)
        for b in range(1, b_sz):
            x_setup(b)
        for b in range(1, b_sz):
            for t in range(NT):
                main_tile(b, t)

    nc.compile()
    return nc


_CACHE = {}


def _get_module():
    if "nc" not in _CACHE:
        _CACHE["nc"] = build_module()
    return _CACHE["nc"]


def _run(inputs: np.ndarray, masks: np.ndarray, W: np.ndarray, **spmd_kwargs):
    """Run on 8 cores; returns (full output, BassKernelResults)."""
    nc = _get_module()

    x = np.ascontiguousarray(inputs, dtype=np.float32)
    mf = np.ascontiguousarray(masks, dtype=np.float32)
    w_pad = np.zeros((L_PAD, H), dtype=np.float32)
    w_pad[:L] = W

    in_maps = [
        {"x": x, "m": mf, "w": np.ascontiguousarray(w_pad[c * LSH : (c + 1) * LSH])}
        for c in range(N_CORES)
    ]
    res = bass_utils.run_bass_kernel_spmd(
        nc, in_maps, core_ids=list(range(N_CORES)), **spmd_kwargs
    )
    out = np.concatenate([res.results[c]["o"] for c in range(N_CORES)], axis=1)
    return np.ascontiguousarray(out[:, :L, :]), res


def kernel(inputs: np.ndarray, masks: np.ndarray, W: np.ndarray) -> np.ndarray:
    out, _ = _run(inputs, masks, W)
    return out
